# revision 44
# baseline (speedup 1.0000x reference)
"""Trainium2 Bass kernel for an 8-head self-attention block (MHA).

Problem: x[2, 4096, 512], 8 heads x 64 dims, torch-Linear q/k/v/o projections,
softmax attention, residual:  out = softmax(q k^T / 8) v @ Wo^T + bo + x.

Sharding (8 NeuronCores, no collectives): core c handles batch b = c // 4 and
query rows (c % 4) * 1024 ... + 1024, for ALL heads.  K/V for the full
sequence are computed on every core of a batch group (projections are cheap
relative to attention), so the output projection is fully local to a core.

The host passes x^T and pre-transposed weights in bf16 (matmul inputs are
bf16 everywhere - fp32 matmuls lower to two HI/LO passes on TRN2 and run
~3x slower; all accumulation/softmax stays f32):
  - kT[f, s] / qT[f, q] bf16 in SBUF, qT pre-scaled by 1/sqrt(64), both
    computed on PE from streamed x^T strips (k and v share the strips)
  - scores^T chunk [s=128, q=1024] = kT_sl.T @ qT_sl on PE (f32 psum)
  - exp on ACT -> P~ bf16 (no max subtraction: scores are O(1) here)
  - PV in natural orientation: lhsT = P~ [s=128, q=128], rhs = [V | 1]
    [s=128, 65] -> accumulates o[q, 65] per q-chunk, softmax denominator
    landing in psum column 64; software-pipelined one chunk behind the
    scores so PE never waits on the exp
  - normalize = per-partition reciprocal + tensor_scalar multiply (both
    cheap on DVE), staged through SBUF so the psum bank frees instantly;
    deferred into the next head's instruction stream
  - o tiles are PE-transposed (identity matmul) into oT[f, q] for the
    output projection; out bias is folded into the residual host-side.
"""

import numpy as np

B = 2
S = 4096
E = 512
H = 8
D = 64
P = 128
EC = E // P          # 4 e-chunks
FC = E // P          # 4 f-chunks
NJ = S // P          # 32 s-chunks
QR = S // 4          # 1024 query rows per core
NQS = QR // 512      # 2 query strips of 512
NKS = S // 512       # 8 s-strips of 512

_CACHE = {}


def _build_nc():
    import concourse.bass as bass
    import concourse.tile as tile
    from concourse import bacc, mybir

    f32 = mybir.dt.float32
    bf16 = mybir.dt.bfloat16
    AFT = mybir.ActivationFunctionType
    Alu = mybir.AluOpType

    nc = bacc.Bacc("TRN2", target_bir_lowering=False, debug=False, num_devices=8)

    xT_d = nc.declare_dram_parameter("xT", [E, S], bf16, isOutput=False)
    xqT_d = nc.declare_dram_parameter("xqT", [E, QR], bf16, isOutput=False)
    xres_d = nc.declare_dram_parameter("xres", [QR, E], f32, isOutput=False)
    wqT_d = nc.declare_dram_parameter("wqT", [E, E], bf16, isOutput=False)
    wkT_d = nc.declare_dram_parameter("wkT", [E, E], bf16, isOutput=False)
    wvT_d = nc.declare_dram_parameter("wvT", [E, E], bf16, isOutput=False)
    woT_d = nc.declare_dram_parameter("woT", [E, E], bf16, isOutput=False)
    bq_d = nc.declare_dram_parameter("bq", [P, FC], f32, isOutput=False)
    bk_d = nc.declare_dram_parameter("bk", [P, FC], f32, isOutput=False)
    bv_d = nc.declare_dram_parameter("bv", [E], f32, isOutput=False)
    ident_d = nc.declare_dram_parameter("ident", [P, P], bf16, isOutput=False)
    out_d = nc.declare_dram_parameter("out", [QR, E], f32, isOutput=True)

    with tile.TileContext(nc) as tc:
        with tc.tile_pool(name="const", bufs=1) as const, \
             tc.tile_pool(name="persist", bufs=1) as persist:

            # ---- constants that live for the whole kernel ----
            wo_sb = const.tile([P, EC, E], bf16)
            bq_sb = const.tile([P, FC], f32)
            bk_sb = const.tile([P, FC], f32)
            bv_sb = const.tile([P, E], f32)
            # identity for PE transposes (loaded after the projection
            # phase - see below - to keep startup DMA on the critical path)
            ident_sb = const.tile([P, P], bf16)
            # residual rows (+ output bias, folded host-side)
            xres_sb = const.tile([P, QR // P, E], f32)

            # ---- persistent activations ----
            kT_sb = persist.tile([P, FC, S], bf16)           # 32 KB/p
            qT_sb = persist.tile([P, FC, QR], bf16)          # 8 KB/p
            v_sb = persist.tile([P, NJ, H, 65], bf16)        # 32.5 KB/p
            oT_sb = persist.tile([P, FC, QR], bf16)          # 8 KB/p

            # constant-1 columns (softmax denominator trick)
            nc.vector.memset(v_sb[:, :, :, 64:65], 1.0)

            # ======= phases B (projections) + C (attention), shared =======
            # pools so Tile can overlap the tail of B with the start of C
            # (a separate psum pool per phase would serialize on the psum
            # stack allocator)
            NQC = QR // P  # 8 query chunks of 128
            with tc.tile_pool(name="wpool", bufs=1) as wpool, \
                 tc.tile_pool(name="xtp", bufs=3) as xtp, \
                 tc.tile_pool(name="work", bufs=4) as work, \
                 tc.tile_pool(name="opool", bufs=2) as opool, \
                 tc.tile_pool(name="ps_sc", bufs=3, space="PSUM") as ps_sc, \
                 tc.tile_pool(name="ps_pv", bufs=1, space="PSUM") as ps_pv:

                wq_sb = wpool.tile([P, EC, E], bf16)
                wk_sb = wpool.tile([P, EC, E], bf16)
                wv_sb = wpool.tile([P, EC, E], bf16)
                # per-e-chunk loads so the first matmul only waits for
                # the first 128 rows of Wq rather than the whole tensor
                for t, d in ((wq_sb, wqT_d), (wk_sb, wkT_d), (wv_sb, wvT_d)):
                    for e in range(EC):
                        nc.sync.dma_start(
                            out=t[:, e, :], in_=d[e * P:(e + 1) * P, :])
                nc.sync.dma_start(out=bq_sb[:], in_=bq_d[:])
                nc.sync.dma_start(out=bk_sb[:], in_=bk_d[:])
                nc.sync.dma_start(
                    out=bv_sb[:],
                    in_=bass.AP(tensor=bv_d, offset=0, ap=[[0, P], [1, E]]))

                # B2: qT[f, q] = (Wq @ xq^T + bq) / 8 (first: C needs it all)
                for qs in range(NQS):
                    qsl = slice(qs * 512, (qs + 1) * 512)
                    xq = xtp.tile([P, EC, 512], bf16, tag="xt")
                    for e in range(EC):
                        nc.sync.dma_start(
                            out=xq[:, e, :], in_=xqT_d[e * P:(e + 1) * P, qsl])
                    for f in range(FC):
                        pq = ps_sc.tile([P, 512], f32, tag="sc", name="pq")
                        for e in range(EC):
                            nc.tensor.matmul(
                                pq[:], wq_sb[:, e, f * P:(f + 1) * P],
                                xq[:, e, :], start=(e == 0), stop=(e == EC - 1),
                                skip_group_check=True)
                        nc.vector.tensor_scalar(
                            qT_sb[:, f, qsl], pq[:], bq_sb[:, f:f + 1],
                            float(1.0 / np.sqrt(D)), Alu.add, Alu.mult)

                # ---- phase C: attention ----
                # PV runs in "natural" orientation: lhsT = P~ slice [s=128,
                # q=128], rhs = [V | 1] [s=128, 65] -> psum o[q, 65].  That
                # streams 65 columns per (qchunk, j) instead of 1024, puts
                # the softmax denominator in a psum COLUMN (normalization is
                # a cheap per-partition tensor_scalar), and the small o tiles
                # are PE-transposed into the oT layout phase D needs.

                def emit_normalize(stg, h, also_d=False, qcs=None):
                    fc = h // 2
                    fr = (h % 2) * 64
                    qcs = range(NQC) if qcs is None else qcs
                    rcp = opool.tile([P, NQC, 1], f32, tag="rcp", name="rcp",
                                     bufs=2)
                    nc.vector.reciprocal(rcp[:, qcs[0]:qcs[-1] + 1, :],
                                         stg[:, qcs[0]:qcs[-1] + 1, 64:65])
                    o_sb = opool.tile([P, NQC, 64], bf16, tag="o", name="o_sb",
                                      bufs=2)
                    for qc in qcs:
                        nc.vector.tensor_scalar_mul(
                            o_sb[:, qc, :], stg[:, qc, 0:64], rcp[:, qc, :])
                    for qc in qcs:
                        # transpose [128 q, 64 d] -> [64 d, 128 q] on PE,
                        # directly at the head's partition base
                        tp = ps_sc.tile([P, P], bf16, tag="sc", name="tp")
                        nc.tensor.transpose(tp[fr:fr + 64, :], o_sb[:, qc, :],
                                            ident_sb[:])
                        nc.vector.tensor_copy(
                            oT_sb[fr:fr + 64, fc, qc * P:(qc + 1) * P],
                            tp[fr:fr + 64, :])
                        if also_d:
                            # last head: output projection for this q-chunk
                            # follows immediately (all other heads' oT pieces
                            # already landed), overlapping phase D with the
                            # tail of attention
                            po = ps_sc.tile([P, E], f32, tag="sc", name="po")
                            for e in range(EC):
                                nc.tensor.matmul(
                                    po[:], oT_sb[:, e, qc * P:(qc + 1) * P],
                                    wo_sb[:, e, :], start=(e == 0),
                                    stop=(e == EC - 1), skip_group_check=True)
                            ot = opool.tile([P, E], f32, tag="ot", name="ot")
                            nc.vector.tensor_add(ot[:], po[:],
                                                 xres_sb[:, qc, :])
                            nc.sync.dma_start(
                                out=out_d[qc * P:(qc + 1) * P, :], in_=ot[:])

                def emit_head_chunk(h, j, pvp, prev_pt):
                    fc = h // 2
                    fr = (h % 2) * 64
                    # scores^T chunk [s=128, q=1024] (two 512 halves)
                    sc = ps_sc.tile([P, QR], f32, tag="sc", name="sc")
                    for hf in range(QR // 512):
                        hsl = slice(hf * 512, (hf + 1) * 512)
                        nc.tensor.matmul(
                            sc[:, hsl],
                            kT_sb[fr:fr + 64, fc, j * P:(j + 1) * P],
                            qT_sb[fr:fr + 64, fc, hsl],
                            start=True, stop=True, skip_group_check=True)
                    pt = work.tile([P, QR], bf16, tag="pt", name="pt")
                    nc.scalar.activation(pt[:], sc[:], AFT.Exp)
                    # software pipeline: PV for chunk j-1 is emitted after the
                    # scores matmuls of chunk j so PE never waits on the exp
                    # of the chunk it just produced
                    if prev_pt is not None:
                        for qc in range(NQC):
                            # a start=True matmul clears its whole psum BANK's
                            # has_written bits, so only the first region per
                            # bank (qc 0 and 4) sets it; the other regions'
                            # first writes then overwrite stale data instead
                            # of accumulating onto it
                            nc.tensor.matmul(
                                pvp[:, qc, 0:65],
                                prev_pt[:, qc * P:(qc + 1) * P],
                                v_sb[:, j - 1, h, :],
                                start=(j - 1 == 0 and qc % 4 == 0),
                                stop=False, skip_group_check=True)
                    return pt

                def finish_head(h, pvp, prev_pt):
                    for qc in range(NQC):
                        nc.tensor.matmul(
                            pvp[:, qc, 0:65], prev_pt[:, qc * P:(qc + 1) * P],
                            v_sb[:, NJ - 1, h, :], start=False, stop=True,
                            skip_group_check=True)
                    # staging copies (one per psum bank) free the single psum
                    # buffer almost immediately
                    stg = opool.tile([P, NQC, 65], f32, tag="stg", name="stg")
                    nc.vector.tensor_copy(stg[:, 0:4], pvp[:, 0:4, 0:65])
                    nc.vector.tensor_copy(stg[:, 4:8], pvp[:, 4:8, 0:65])
                    return (stg, h)

                # B1+B3+head-0 interleaved: kT strips and V chunks come from
                # the same xt tile, and head 0's scores/exp/PV for a strip's
                # four chunks follow immediately, so the ACT exp pipeline
                # starts ~70us earlier and fills projection DMA gaps
                pvp0 = ps_pv.tile([P, NQC, P], f32, tag="pv", name="pvp0")
                pt0 = None
                for strip in range(NKS):
                    ssl = slice(strip * 512, (strip + 1) * 512)
                    xt = xtp.tile([P, EC, 512], bf16, tag="xt")
                    for e in range(EC):
                        nc.sync.dma_start(
                            out=xt[:, e, :], in_=xT_d[e * P:(e + 1) * P, ssl])
                    for f in range(FC):
                        pk = ps_sc.tile([P, 512], f32, tag="sc", name="pk")
                        for e in range(EC):
                            nc.tensor.matmul(
                                pk[:], wk_sb[:, e, f * P:(f + 1) * P],
                                xt[:, e, :], start=(e == 0), stop=(e == EC - 1),
                                skip_group_check=True)
                        nc.vector.tensor_scalar_add(
                            kT_sb[:, f, ssl], pk[:], bk_sb[:, f:f + 1])
                    for k in range(4):
                        j = strip * 4 + k
                        pv = ps_sc.tile([P, E], f32, tag="sc", name="pvx")
                        for e in range(EC):
                            nc.tensor.matmul(
                                pv[:], xt[:, e, k * P:(k + 1) * P],
                                wv_sb[:, e, :], start=(e == 0),
                                stop=(e == EC - 1), skip_group_check=True)
                        pv_v = pv[:].rearrange("p (h d) -> p h d", h=H)
                        bv_v = bv_sb[:].rearrange("p (h d) -> p h d", h=H)
                        nc.vector.tensor_add(v_sb[:, j, :, 0:64], pv_v[:],
                                             bv_v[:])
                    for k in range(4):
                        pt0 = emit_head_chunk(0, strip * 4 + k, pvp0, pt0)
                pending = finish_head(0, pvp0, pt0)

                # tail-only data: loaded now, off the startup critical path
                nc.sync.dma_start(out=ident_sb[:], in_=ident_d[:])
                nc.sync.dma_start(
                    out=wo_sb[:],
                    in_=woT_d.ap().rearrange("(c p) f -> p c f", p=P))
                nc.sync.dma_start(
                    out=xres_sb[:],
                    in_=xres_d.ap().rearrange("(k p) f -> p k f", p=P))

                for h in range(1, H):
                    pvp = ps_pv.tile([P, NQC, P], f32, tag="pv", name="pvp")
                    prev_pt = None
                    for j in range(NJ):
                        prev_pt = emit_head_chunk(h, j, pvp, prev_pt)
                        if pending is not None and j in (8, 20):
                            # deferred: previous head's normalize runs inside
                            # this head's stream, long after its inputs
                            # landed, in two half-blocks to spread the PE
                            # transpose work
                            half = range(NQC // 2) if j == 8 \
                                else range(NQC // 2, NQC)
                            emit_normalize(*pending, qcs=half)
                            if j == 20:
                                pending = None
                    pending = finish_head(h, pvp, prev_pt)
                emit_normalize(*pending, also_d=True)

    nc.compile()
    return nc


def _get_nc():
    if "nc" not in _CACHE:
        _CACHE["nc"] = _build_nc()
    return _CACHE["nc"]


def run_spmd(in_maps, **kw):
    from concourse.bass_utils import run_bass_kernel_spmd
    nc = _get_nc()
    return run_bass_kernel_spmd(nc, in_maps, list(range(8)), **kw)


def make_in_maps(x, Wq, bq, Wk, bk, Wv, bv, Wo, bo):
    import ml_dtypes
    bf = ml_dtypes.bfloat16
    x = np.asarray(x, dtype=np.float32)
    f32c = lambda a: np.ascontiguousarray(np.asarray(a, dtype=np.float32))
    bfc = lambda a: np.ascontiguousarray(
        np.asarray(a, dtype=np.float32).astype(bf))
    wqT = bfc(np.asarray(Wq).T)
    wkT = bfc(np.asarray(Wk).T)
    wvT = bfc(np.asarray(Wv).T)
    woT = bfc(np.asarray(Wo).T)
    bq_r = f32c(np.asarray(bq).reshape(FC, P).T)
    bk_r = f32c(np.asarray(bk).reshape(FC, P).T)
    bv_a = f32c(bv)
    bo_a = np.asarray(bo, dtype=np.float32)
    ident = np.eye(P, dtype=np.float32).astype(bf)
    xT = [bfc(x[b].T) for b in range(B)]

    in_maps = []
    for c in range(8):
        b, r = c // 4, c % 4
        in_maps.append({
            "xT": xT[b],
            "xqT": np.ascontiguousarray(xT[b][:, r * QR:(r + 1) * QR]),
            # output bias folded into the residual tile (host-side, free)
            "xres": f32c(x[b, r * QR:(r + 1) * QR] + bo_a),
            "wqT": wqT, "wkT": wkT, "wvT": wvT, "woT": woT,
            "bq": bq_r, "bk": bk_r, "bv": bv_a,
            "ident": ident,
        })
    return in_maps


def assemble(results):
    out = np.empty((B, S, E), dtype=np.float32)
    for c in range(8):
        b, r = c // 4, c % 4
        out[b, r * QR:(r + 1) * QR] = results[c]["out"]
    return out


def kernel(x, Wq, bq, Wk, bk, Wv, bv, Wo, bo):
    in_maps = make_in_maps(x, Wq, bq, Wk, bk, Wv, bv, Wo, bo)
    res = run_spmd(in_maps)
    return assemble(res.results)
